# revision 5
# baseline (speedup 1.0000x reference)
"""3-layer GCN (GCNConv + residual + relu, global add pool, MLP softmax) on 8
Trainium2 NeuronCores.

Sharding: nodes by range; EDGES PARTITIONED BY SOURCE core (scatter mode).
Per layer, each core:
  phase A: xw = (dinv*h) @ Wg computed node-major into a LOCAL DRAM table
    (no collective needed — every message's source row is core-local).
  phase B: dma_gather per-edge source rows from the local table (int16
    indices fit trivially), segment-sum them into per-128-dst-window PSUM
    accumulators via one-hot matmuls (fp16 one-hots built on the DVE at its
    2x rate from dstrel slabs vs a host-built "staircase" constant;
    window-crossing extras use host-prepared shifted dstrel columns).
    Destination windows span ALL 8 node shards (784 windows); per-superblock
    PSUM is drained to an fp8 staging tile and DMA'd into a full-size
    [8*128 feat-rows x 12544 node-cols] partial-aggregate table.
  ReduceScatter (cheap: the collective cost model charges by OUTPUT size,
    and the RS output is 1/8 of the table) sums partials across cores and
    leaves each core its own shard, feature-major — exactly the epilogue
    layout. The layer is split into 2 column halves with one RS each so the
    first RS overlaps the second half's compute.
  epilogue: h = relu(h + dinv*agg + bg), chunked, immediately followed by
    the next layer's phase A for those columns (overlaps the second RS).
Pooled [64,128] partials are AllGathered and summed locally; the tiny
classifier is replicated. All cores run the IDENTICAL program; per-core
variation lives entirely in data (gather indices, sel values, padding).
"""
import numpy as np
import ml_dtypes

import concourse.bacc as bacc
import concourse.bass as bass
import concourse.mybir as mybir
import concourse.tile as tile
from concourse.bass_utils import run_bass_kernel_spmd

NCORES = 8
G = 64     # graphs in batch (pooled rows)
C = 2      # classes
SBW = 7    # dst windows per superblock (psum granularity); 49 = 7*7
NSPLIT = 2  # column halves per layer, one ReduceScatter each
SLAB = 32  # sel tiles built per is_equal op

bf16 = ml_dtypes.bfloat16
_cache = {}


def _ceil(a, b):
    return -(-a // b)


# --------------------------------------------------------------------------
# host preprocessing
# --------------------------------------------------------------------------
def _preprocess(x, edge_index, batch):
    N, D = x.shape
    assert D == 128 and N % NCORES == 0
    NLOC = N // NCORES
    NPAD = _ceil(NLOC, 128) * 128
    NW = NPAD // 128
    assert NW % NSPLIT == 0
    WSPL = NW // NSPLIT
    sbw = min(SBW, WSPL)
    assert WSPL % sbw == 0
    NWF = NCORES * NW           # dst windows across all shards
    NSBF = NWF // sbw

    src = np.asarray(edge_index[0], np.int64)
    dst = np.asarray(edge_index[1], np.int64)
    deg = np.bincount(dst, minlength=N).astype(np.float64) + 1.0
    dinv = (deg ** -0.5).astype(np.float32)

    loops = np.arange(N, dtype=np.int64)
    src_f = np.concatenate([src, loops])
    dst_f = np.concatenate([dst, loops])

    mc = src_f // NLOC                  # owning core (message source)
    sl = src_f - mc * NLOC              # gather index into local table
    cd = dst_f // NLOC
    dl = dst_f - cd * NLOC
    wl = dl // 128                      # local dst window 0..NW-1
    sp = wl // WSPL                     # column split
    # stream window order: split -> dst core -> local window
    swi = sp * (NCORES * WSPL) + cd * WSPL + (wl - sp * WSPL)
    drel = dl - wl * 128

    cnt = np.bincount(mc * NWF + swi, minlength=NCORES * NWF
                      ).reshape(NCORES, NWF)
    cap = cnt.max(axis=0)               # identical stream on every core

    # slot stream: windows in swi order, superblocks padded to x128
    win_off = np.zeros(NWF, np.int64)
    sb_tiles = []                       # (tile_off, n_tiles) per superblock
    slot_w_list = []
    so = 0
    for sb in range(NSBF):
        t0 = so // 128
        for j in range(sbw):
            w = sb * sbw + j
            win_off[w] = so
            slot_w_list.append(np.full(int(cap[w]), w, np.int64))
            so += int(cap[w])
        pad = (-so) % 128
        if pad:
            slot_w_list.append(np.full(pad, -1, np.int64))
            so += pad
        sb_tiles.append((t0, so // 128 - t0))
    SLOTS = so
    NT = SLOTS // 128
    slot_w = np.concatenate(slot_w_list)

    # tiles -> MM list (tile, stream window, iota_k); start/stop per window
    first_w = np.empty(NT, np.int64)
    mm_list = []
    for t in range(NT):
        ws_here = slot_w[t * 128:(t + 1) * 128]
        ws_u = np.unique(ws_here[ws_here >= 0])
        fw = int(ws_u[0]) if len(ws_u) else int(slot_w[t * 128 - 1])
        first_w[t] = fw
        for w_ in ws_u:
            k = int(w_ - fw)
            assert 0 <= k < sbw
            mm_list.append((t, int(w_), k))
    firstmm = np.full(NWF, -1, np.int64)
    lastmm = np.full(NWF, -1, np.int64)
    for i, (t, w_, k) in enumerate(mm_list):
        if firstmm[w_] < 0:
            firstmm[w_] = i
        lastmm[w_] = i
    assert (firstmm >= 0).all(), "every window needs at least one MM"
    ex_cols = []
    mm_flags = []
    for i, (t, w_, k) in enumerate(mm_list):
        e = -1
        if k > 0:
            e = len(ex_cols)
            ex_cols.append((t, k))
        mm_flags.append((t, w_, k, i == firstmm[w_], i == lastmm[w_], e))
    NEX = len(ex_cols)
    NEXP = _ceil(max(NEX, 1), SLAB) * SLAB

    # gather groups: per (split, dst core) block of sbs -> chunks of <=4
    groups = []   # dicts: t0, nt, sbs [(sb, wl0, cd, tile_off, n_tiles)]
    nsb_blk = WSPL // sbw
    chunks = [(i, min(i + 4, nsb_blk)) for i in range(0, nsb_blk, 4)]
    if len(chunks) >= 2 and chunks[-1][1] - chunks[-1][0] == 1:
        a, b = chunks[-2]
        chunks[-2] = (a, b - 1)
        chunks[-1] = (b - 1, nsb_blk)
    for spc in range(NSPLIT):
        for cdc in range(NCORES):
            b0 = (spc * NCORES + cdc) * nsb_blk
            for lo, hi in chunks:
                sbs = list(range(b0 + lo, b0 + hi))
                t0 = sb_tiles[sbs[0]][0]
                nt = sum(sb_tiles[s][1] for s in sbs)
                info = []
                for s in sbs:
                    sbl = s - b0
                    wl0 = spc * WSPL + sbl * sbw
                    info.append((s, wl0, cdc, sb_tiles[s][0], sb_tiles[s][1]))
                groups.append(dict(sp=spc, cd=cdc, t0=t0, nt=nt, sbs=info))

    # per-core slot placement
    order = np.lexsort((swi, mc))
    mc_s = mc[order]
    keyall = mc_s * NWF + swi[order]
    starts = np.r_[0, np.flatnonzero(np.diff(keyall)) + 1]
    gid = np.zeros(len(keyall), np.int64)
    gid[starts[1:]] = 1
    gid = np.cumsum(gid)
    pos = np.arange(len(keyall)) - starts[gid]
    slot = win_off[swi[order]] + pos
    assert (pos < cap[swi[order]]).all()

    gidx_all = np.zeros((NCORES, SLOTS), np.int16)
    dstrel_all = np.full((NCORES, SLOTS), -1.0, np.float32)
    gidx_all[mc_s, slot] = sl[order].astype(np.int16)
    dstrel_all[mc_s, slot] = ((swi[order] - first_w[slot // 128]) * 128
                              + drel[order]).astype(np.float32)
    assert (dstrel_all[mc_s, slot] >= 0).all()
    assert dstrel_all.max() < sbw * 128

    gidx_dev = np.tile(
        gidx_all.reshape(NCORES, SLOTS // 16, 16).transpose(0, 2, 1), (1, 8, 1)
    ).copy()                                        # [8, 128, SLOTS//16]
    dstrel_dev = dstrel_all.reshape(NCORES, NT, 128).transpose(0, 2, 1).copy()

    dstrel_ex_dev = np.full((NCORES, 128, NEXP), -1000.0, np.float32)
    for e, (t, k) in enumerate(ex_cols):
        dstrel_ex_dev[:, :, e] = dstrel_dev[:, :, t] - 128.0 * k

    batch = np.asarray(batch, np.int64)
    brel = np.full((NCORES, NPAD), -1.0, np.float32)
    for cc in range(NCORES):
        brel[cc, :NLOC] = batch[cc * NLOC:(cc + 1) * NLOC]
    batchrel_dev = brel.reshape(NCORES, NW, 128).transpose(0, 2, 1).copy()

    x = np.asarray(x, np.float32)
    xt_dev = np.zeros((NCORES, 128, NPAD), bf16)
    dinvT_dev = np.zeros((NCORES, 128, NPAD), bf16)
    for cc in range(NCORES):
        xl = x[cc * NLOC:(cc + 1) * NLOC]
        xt_dev[cc, :, :NLOC] = xl.T.astype(bf16)
        dv = np.zeros(NPAD, np.float32)
        dv[:NLOC] = dinv[cc * NLOC:(cc + 1) * NLOC]
        dinvT_dev[cc] = np.broadcast_to(dv.astype(bf16), (128, NPAD))

    meta = dict(N=N, NLOC=NLOC, NPAD=NPAD, NW=NW, WSPL=WSPL, NWF=NWF,
                NT=NT, SLOTS=SLOTS, groups=groups, mm_flags=mm_flags,
                NEX=NEX, NEXP=NEXP, SBWE=sbw)
    data = dict(gidx=gidx_dev, dstrel=dstrel_dev, batchrel=batchrel_dev,
                xt=xt_dev, dinvt=dinvT_dev, dstrel_ex=dstrel_ex_dev)
    return meta, data


# --------------------------------------------------------------------------
# device program
# --------------------------------------------------------------------------
def _build(meta, L):
    f32 = mybir.dt.float32
    b16 = mybir.dt.bfloat16
    f16 = mybir.dt.float16
    fp8 = mybir.dt.float8e4
    i16 = mybir.dt.int16
    NPAD, NW, WSPL = meta["NPAD"], meta["NW"], meta["WSPL"]
    NT, SLOTS = meta["NT"], meta["SLOTS"]
    SBWE = meta["SBWE"]
    NEXP = meta["NEXP"]
    groups, mm_flags = meta["groups"], meta["mm_flags"]
    rg = [list(range(NCORES))]
    mm_by_tile = {}
    for (t, w_, k, st_f, sp_f, e) in mm_flags:
        mm_by_tile.setdefault(t, []).append((w_, k, st_f, sp_f, e))

    nc = bacc.Bacc("TRN2", target_bir_lowering=False, debug=False,
                   num_devices=NCORES)
    d_xt = nc.dram_tensor("xt", [128, NPAD], b16, kind="ExternalInput")
    d_dinvt = nc.dram_tensor("dinvt", [128, NPAD], b16, kind="ExternalInput")
    d_gidx = nc.dram_tensor("gidx", [128, SLOTS // 16], i16, kind="ExternalInput")
    d_dstrel = nc.dram_tensor("dstrel", [128, NT], f16, kind="ExternalInput")
    d_batchrel = nc.dram_tensor("batchrel", [128, NW], f16, kind="ExternalInput")
    d_w0 = nc.dram_tensor("w0", [128, 128], b16, kind="ExternalInput")
    d_wg = nc.dram_tensor("wg", [L, 128, 128], b16, kind="ExternalInput")
    d_wc1 = nc.dram_tensor("wc1", [128, 128], b16, kind="ExternalInput")
    d_wc2 = nc.dram_tensor("wc2", [128, C], b16, kind="ExternalInput")
    d_b0 = nc.dram_tensor("b0", [128, 1], f32, kind="ExternalInput")
    d_bg = nc.dram_tensor("bg", [L, 128, 1], f32, kind="ExternalInput")
    d_bc1 = nc.dram_tensor("bc1", [128, 1], f32, kind="ExternalInput")
    d_bc2m = nc.dram_tensor("bc2m", [G, C], f32, kind="ExternalInput")
    d_stair = nc.dram_tensor("stair", [128, 128 * SLAB], f16,
                             kind="ExternalInput")
    d_dstrel_ex = nc.dram_tensor("dstrel_ex", [128, NEXP], f16,
                                 kind="ExternalInput")
    d_id128 = nc.dram_tensor("id128", [128, 128], b16, kind="ExternalInput")
    d_idg = nc.dram_tensor("idg", [G, G], b16, kind="ExternalInput")
    d_out = nc.dram_tensor("out", [G, C], f32, kind="ExternalOutput")

    xw_loc = [nc.dram_tensor(f"xw_loc{l}", [NPAD, 128], b16)
              for l in range(L)]
    CSPL = WSPL * 128
    rs_in = [[nc.dram_tensor(f"rs_in{l}_{s}", [NCORES * 128, CSPL], fp8)
              for s in range(NSPLIT)] for l in range(L)]
    rs_out = [[nc.dram_tensor(f"rs_out{l}_{s}", [128, CSPL], fp8)
               for s in range(NSPLIT)] for l in range(L)]
    pool_in = nc.dram_tensor("pool_in", [G, 128], b16)
    pool_out = nc.dram_tensor("pool_out", [NCORES * G, 128], b16,
                              addr_space="Shared")

    Relu = mybir.ActivationFunctionType.Relu
    Exp = mybir.ActivationFunctionType.Exp
    Copy = mybir.ActivationFunctionType.Copy
    AT = mybir.AluOpType

    with tile.TileContext(nc) as tc:
        with (
            tc.tile_pool(name="state", bufs=1) as state,
            tc.tile_pool(name="wpool", bufs=1) as wpool,
            tc.tile_pool(name="xin", bufs=3) as xinp,
            tc.tile_pool(name="xws", bufs=3) as xwsp,
            tc.tile_pool(name="xwn", bufs=3) as xwnp,
            tc.tile_pool(name="gix", bufs=4) as gixp,
            tc.tile_pool(name="gbf", bufs=2) as gbfp,
            tc.tile_pool(name="sel", bufs=2) as selp,
            tc.tile_pool(name="stg", bufs=3) as stgp,
            tc.tile_pool(name="epi", bufs=6) as epip,
            tc.tile_pool(name="psxw", bufs=2, space="PSUM") as psxw,
            tc.tile_pool(name="pstr", bufs=2, space="PSUM") as pstr,
            tc.tile_pool(name="pswin", bufs=2, space="PSUM") as pswin,
        ):
            # ---- persistent state + constants ----
            h = state.tile([128, NPAD], b16, tag="h")
            dinvT = state.tile([128, NPAD], b16, tag="dinvT")
            dstrel = state.tile([128, NT], f16, tag="dstrel")
            gidxs = state.tile([128, SLOTS // 16], i16, tag="gidxs")
            nc.gpsimd.dma_start(gidxs[:], d_gidx[:])

            w0 = wpool.tile([128, 128], b16, tag="w0")
            nc.sync.dma_start(w0[:], d_w0[:])
            b0 = wpool.tile([128, 1], f32, tag="b0")
            nc.sync.dma_start(b0[:], d_b0[:])
            wg = wpool.tile([128, L, 128], b16, tag="wg")
            nc.sync.dma_start(wg[:], d_wg.rearrange("l p f -> p l f"))
            nc.gpsimd.dma_start(dinvT[:], d_dinvt[:])
            nc.gpsimd.dma_start(dstrel[:], d_dstrel[:])
            wc1 = wpool.tile([128, 128], b16, tag="wc1")
            nc.scalar.dma_start(wc1[:], d_wc1[:])
            wc2 = wpool.tile([128, C], b16, tag="wc2")
            nc.scalar.dma_start(wc2[:], d_wc2[:])
            bg = wpool.tile([128, L], f32, tag="bg")
            nc.scalar.dma_start(bg[:], d_bg.rearrange("l p o -> p (l o)"))
            bc1 = wpool.tile([128, 1], f32, tag="bc1")
            nc.scalar.dma_start(bc1[:], d_bc1[:])
            bc2m = wpool.tile([G, C], f32, tag="bc2m")
            nc.scalar.dma_start(bc2m[:], d_bc2m[:])
            stair = wpool.tile([128, 128 * SLAB], f16, tag="stair")
            nc.gpsimd.dma_start(stair[:], d_stair[:])
            dstrel_ex = wpool.tile([128, NEXP], f16, tag="dstrel_ex")
            nc.gpsimd.dma_start(dstrel_ex[:], d_dstrel_ex[:])
            id128 = wpool.tile([128, 128], b16, tag="id128")
            nc.sync.dma_start(id128[:], d_id128[:])
            idg = wpool.tile([G, G], b16, tag="idg")
            nc.scalar.dma_start(idg[:], d_idg[:])
            batchrel = wpool.tile([128, NW], f16, tag="batchrel")
            nc.gpsimd.dma_start(batchrel[:], d_batchrel[:])

            def emit_phaseA_cols(l, c0, c1hi):
                """xw_loc[l] rows [c0,c1hi) (node-major) from current h."""
                while c0 < c1hi:
                    cw = min(512, c1hi - c0)
                    hs = xwsp.tile([128, cw], b16, tag="xws", name="hs")
                    nc.vector.tensor_tensor(out=hs[:], in0=h[:, c0:c0 + cw],
                                            in1=dinvT[:, c0:c0 + cw],
                                            op=AT.mult)
                    ps = psxw.tile([128, cw], f32, tag="psxw", name="ps")
                    for j in range(cw // 128):
                        nc.tensor.matmul(ps[:, j * 128:(j + 1) * 128],
                                         lhsT=hs[:, j * 128:(j + 1) * 128],
                                         rhs=wg[:, l, :],
                                         start=True, stop=True)
                    xwn = xwnp.tile([128, cw // 128, 128], b16, tag="xwn",
                                    name="xwn")
                    nc.scalar.activation(
                        out=xwn[:],
                        in_=bass.AP(ps.tensor, ps[:].offset,
                                    [ps[:].ap[0], [128, cw // 128], [1, 128]]),
                        func=Copy)
                    nc.sync.dma_start(
                        bass.AP(xw_loc[l], c0 * 128,
                                [[128, 128], [128 * 128, cw // 128], [1, 128]]),
                        xwn[:])
                    c0 += cw

            # ---- stage 1: h = relu(W0.T @ xT + b0) ----
            nchunks = _ceil(NPAD, 512)
            for kk in range(nchunks):
                c0 = kk * 512
                cw = min(512, NPAD - c0)
                xts = xinp.tile([128, cw], b16, tag="xts", name="xts")
                nc.sync.dma_start(xts[:], d_xt[:, c0:c0 + cw])
                ps = psxw.tile([128, cw], f32, tag="psxw", name="ps")
                nc.tensor.matmul(ps[:], lhsT=w0[:], rhs=xts[:],
                                 start=True, stop=True)
                nc.scalar.activation(out=h[:, c0:c0 + cw], in_=ps[:],
                                     func=Relu, bias=b0[:])
            emit_phaseA_cols(0, 0, NPAD)

            # ---- GCN layers ----
            gsrc = [bass.AP(xw_loc[l], 0, [[128, NPAD], [1, 128]])
                    for l in range(L)]
            for l in range(L):
                sel_tiles = {}
                ex_tiles = {}
                ps_sb = {}

                def get_sel(t):
                    s = t // SLAB
                    if s not in sel_tiles:
                        t0 = s * SLAB
                        tn = min(SLAB, NT - t0)
                        st = selp.tile([128, 128, tn], f16, tag="sel",
                                       name="st")
                        in0 = bass.AP(dstrel.tensor,
                                      dstrel[:, t0:t0 + tn].offset,
                                      [dstrel[:].ap[0], [0, 128], [1, tn]])
                        in1 = bass.AP(stair.tensor, stair[:].offset,
                                      [stair[:].ap[0], [SLAB, 128], [1, tn]])
                        nc.vector.tensor_tensor(out=st[:], in0=in0, in1=in1,
                                                op=AT.is_equal)
                        sel_tiles.clear()
                        sel_tiles[s] = (st, tn)
                    st, tn = sel_tiles[s]
                    return st, t - s * SLAB, tn

                def get_ex(e):
                    s = e // SLAB
                    if s not in ex_tiles:
                        e0 = s * SLAB
                        en = min(SLAB, NEXP - e0)
                        sx = selp.tile([128, 128, en], f16, tag="selx",
                                       name="sx", bufs=2)
                        in0 = bass.AP(dstrel_ex.tensor,
                                      dstrel_ex[:, e0:e0 + en].offset,
                                      [dstrel_ex[:].ap[0], [0, 128], [1, en]])
                        in1 = bass.AP(stair.tensor, stair[:].offset,
                                      [stair[:].ap[0], [SLAB, 128], [1, en]])
                        nc.vector.tensor_tensor(out=sx[:], in0=in0, in1=in1,
                                                op=AT.is_equal)
                        ex_tiles.clear()
                        ex_tiles[s] = (sx, en)
                    sx, en = ex_tiles[s]
                    return sx, e - s * SLAB, en

                def emit_group(gr):
                    t0g, ntg = gr["t0"], gr["nt"]
                    slots = ntg * 128
                    so = t0g * 128
                    gb = gbfp.tile([128, ntg, 128], b16, tag="gbf")
                    nc.gpsimd.dma_gather(
                        gb[:], gsrc[l], gidxs[:, so // 16:(so + slots) // 16],
                        slots, slots, 128, elem_step=128, single_packet=False)
                    for (sbi, wl0, cdc, sbt0, sbnt) in gr["sbs"]:
                        if sbnt == 0:
                            continue
                        pst = pswin.tile([128, SBWE * 128], f32,
                                         name="pswin_t", tag="pswin")
                        ps_sb[sbi] = pst
                        w0s = sbi * SBWE
                        for ti in range(sbt0, sbt0 + sbnt):
                            st, si, tn = get_sel(ti)
                            for (w_, k, st_f, sp_f, e) in mm_by_tile.get(ti, []):
                                wr = w_ - w0s
                                if k == 0:
                                    rhs = bass.AP(
                                        st.tensor, st[:].offset + si,
                                        [st[:].ap[0], [tn, 128]])
                                else:
                                    sx, se, en = get_ex(e)
                                    rhs = bass.AP(
                                        sx.tensor, sx[:].offset + se,
                                        [sx[:].ap[0], [en, 128]])
                                nc.tensor.matmul(
                                    pst[:, wr * 128:(wr + 1) * 128],
                                    lhsT=gb[:, ti - t0g, :], rhs=rhs,
                                    start=bool(st_f), stop=bool(sp_f))
                        # drain superblock -> fp8 staging -> rs_in slice
                        stg = stgp.tile([128, SBWE * 128], fp8, tag="stg",
                                        name="stg")
                        nc.scalar.activation(out=stg[:], in_=pst[:], func=Copy)
                        spg = gr["sp"]
                        nc.sync.dma_start(
                            bass.AP(rs_in[l][spg],
                                    cdc * 128 * CSPL
                                    + (wl0 - spg * WSPL) * 128,
                                    [[CSPL, 128], [1, SBWE * 128]]),
                            stg[:])

                # pool state for last layer
                pool_st = {}
                if l == L - 1:
                    pool_st["psp"] = psxw.tile([G, 128], f32, tag="psxw",
                                               name="psp")

                def emit_pool(wlo, whi):
                    for a in range(wlo, whi):
                        pstt = pstr.tile([128, 128], b16, tag="pstr",
                                         name="pst2")
                        nc.tensor.transpose(
                            pstt[:], h[:, a * 128:(a + 1) * 128], id128[:])
                        hn = epip.tile([128, 128], b16, tag="hn", name="hn")
                        nc.scalar.activation(out=hn[:], in_=pstt[:], func=Copy)
                        if a % SLAB == 0:
                            a0 = a
                            an = min(SLAB, NW - a0)
                            bsel = selp.tile([128, G, an], f16, tag="sel",
                                             name="bsel")
                            in0 = bass.AP(
                                batchrel.tensor,
                                batchrel[:, a0:a0 + an].offset,
                                [batchrel[:].ap[0], [0, G], [1, an]])
                            in1 = bass.AP(
                                stair.tensor, stair[:].offset,
                                [stair[:].ap[0], [SLAB, G], [1, an]])
                            nc.vector.tensor_tensor(
                                out=bsel[:], in0=in0, in1=in1,
                                op=AT.is_equal)
                            pool_st["bsel"] = (bsel, a0, an)
                        bsel, a0, an = pool_st["bsel"]
                        blhs = bass.AP(bsel.tensor,
                                       bsel[:].offset + (a - a0),
                                       [bsel[:].ap[0], [an, G]])
                        nc.tensor.matmul(pool_st["psp"][:], lhsT=blhs,
                                         rhs=hn[:], start=(a == 0),
                                         stop=(a == NW - 1))

                def emit_epilogue_split(s):
                    """h cols of split s = relu(h + dinv*agg + bg); then
                    next-layer phase A (or pooling) for those columns."""
                    cs0 = s * WSPL * 128
                    cw_total = WSPL * 128
                    CH = 2 * SBWE * 128
                    c0 = cs0
                    while c0 < cs0 + cw_total:
                        cw = min(CH, cs0 + cw_total - c0)
                        agg = epip.tile([128, cw], fp8, tag="agg", name="agg",
                                        bufs=2)
                        nc.sync.dma_start(
                            agg[:], rs_out[l][s][:, c0 - cs0:c0 - cs0 + cw])
                        u = epip.tile([128, cw], b16, tag="u", name="u",
                                      bufs=2)
                        nc.vector.tensor_tensor(out=u[:], in0=agg[:],
                                                in1=dinvT[:, c0:c0 + cw],
                                                op=AT.mult)
                        u2 = epip.tile([128, cw], b16, tag="u2", name="u2",
                                       bufs=2)
                        nc.vector.tensor_tensor(out=u2[:], in0=u[:],
                                                in1=h[:, c0:c0 + cw],
                                                op=AT.add)
                        nc.scalar.activation(out=h[:, c0:c0 + cw], in_=u2[:],
                                             func=Relu, bias=bg[:, l:l + 1])
                        if l + 1 < L:
                            emit_phaseA_cols(l + 1, c0, c0 + cw)
                        else:
                            emit_pool(c0 // 128, (c0 + cw) // 128)
                        c0 += cw

                glist = [gr for gr in groups]
                per_split = len(glist) // NSPLIT
                for s in range(NSPLIT):
                    for gr in glist[s * per_split:(s + 1) * per_split]:
                        emit_group(gr)
                    if s + 1 < NSPLIT:
                        # head of next split before the RS so its desc-gen
                        # isn't blocked behind the collective's SEQ wait
                        emit_group(glist[(s + 1) * per_split])
                    nc.gpsimd.collective_compute(
                        "ReduceScatter", AT.add,
                        ins=[rs_in[l][s][:]], outs=[rs_out[l][s][:]],
                        replica_groups=rg)
                    if s + 1 < NSPLIT:
                        for gr in glist[(s + 1) * per_split + 1:
                                        (s + 2) * per_split]:
                            emit_group(gr)
                # epilogues (+ next phase A / pooling), per split
                for s in range(NSPLIT):
                    emit_epilogue_split(s)

                # ---- global add pool tail + classifier ----
                if l == L - 1:
                    pool_sb = epip.tile([G, 128], b16, tag="poolsb")
                    nc.vector.tensor_copy(out=pool_sb[:], in_=pool_st["psp"][:])
                    nc.sync.dma_start(pool_in[:], pool_sb[:])
                    nc.gpsimd.collective_compute(
                        "AllGather", AT.bypass, ins=[pool_in[:]],
                        outs=[pool_out[:]], replica_groups=rg)
                    pooled_a = epip.tile([G, 128, NCORES], b16, tag="pooleda",
                                         name="pooled_a", bufs=1)
                    nc.sync.dma_start(
                        pooled_a[:],
                        bass.AP(pool_out, 0,
                                [[128, G], [1, 128], [G * 128, NCORES]]))
                    pooled_f = epip.tile([G, 128], f32, tag="pooledf")
                    nc.vector.tensor_reduce(
                        out=bass.AP(pooled_f.tensor, pooled_f[:].offset,
                                    [pooled_f[:].ap[0], [1, 128], [0, 1]]),
                        in_=pooled_a[:], axis=mybir.AxisListType.X, op=AT.add)
                    pooled_b = epip.tile([G, 128], b16, tag="pooledb")
                    nc.vector.tensor_copy(out=pooled_b[:], in_=pooled_f[:])
                    pstp = pstr.tile([128, G], b16, tag="pstr")
                    nc.tensor.transpose(pstp[:], pooled_b[:], idg[:])
                    pooledT = epip.tile([128, G], b16, tag="pooledT")
                    nc.vector.tensor_copy(out=pooledT[:], in_=pstp[:])
                    psz = pstr.tile([128, G], f32, tag="pstr")
                    nc.tensor.matmul(psz[:], lhsT=wc1[:], rhs=pooledT[:],
                                     start=True, stop=True)
                    zt = epip.tile([128, G], b16, tag="zt")
                    nc.scalar.activation(out=zt[:], in_=psz[:], func=Relu,
                                         bias=bc1[:])
                    pslg = pstr.tile([G, C], f32, tag="pstr")
                    nc.tensor.matmul(pslg[:], lhsT=zt[:], rhs=wc2[:],
                                     start=True, stop=True)
                    lg = epip.tile([G, C], f32, tag="lg")
                    nc.vector.tensor_tensor(out=lg[:], in0=pslg[:],
                                            in1=bc2m[:], op=AT.add)
                    mx = epip.tile([G, 1], f32, tag="mx")
                    nc.vector.tensor_reduce(out=mx[:], in_=lg[:],
                                            axis=mybir.AxisListType.X,
                                            op=AT.max)
                    nmx = epip.tile([G, 1], f32, tag="nmx")
                    nc.vector.tensor_scalar_mul(nmx[:], mx[:], -1.0)
                    ex = epip.tile([G, C], f32, tag="ex")
                    nc.scalar.activation(out=ex[:], in_=lg[:], func=Exp,
                                         bias=nmx[:])
                    sm = epip.tile([G, 1], f32, tag="sm")
                    nc.vector.tensor_reduce(out=sm[:], in_=ex[:],
                                            axis=mybir.AxisListType.X,
                                            op=AT.add)
                    rs = epip.tile([G, 1], f32, tag="rs")
                    nc.vector.reciprocal(rs[:], sm[:])
                    prob = epip.tile([G, C], f32, tag="prob")
                    nc.vector.tensor_scalar_mul(prob[:], ex[:], rs[:])
                    nc.sync.dma_start(d_out[:], prob[:])

    nc.compile()
    return nc


# --------------------------------------------------------------------------
# entry point
# --------------------------------------------------------------------------
def kernel(x, edge_index, batch, W0, b0, Wg, bg, Wc1, bc1, Wc2, bc2,
           **extra):
    x = np.asarray(x, np.float32)
    edge_index = np.asarray(edge_index)
    batch = np.asarray(batch)
    W0 = np.asarray(W0, np.float32)
    Wg = np.asarray(Wg, np.float32)
    L = Wg.shape[0]

    key = (x.shape, edge_index.shape,
           hash(edge_index.tobytes()), hash(np.asarray(batch).tobytes()))
    if key not in _cache:
        meta, data = _preprocess(x, edge_index, batch)
        nc = _build(meta, L)
        _cache.clear()
        _cache[key] = (meta, data, nc)
    meta, data, nc = _cache[key]

    stair = np.broadcast_to(
        np.repeat(np.arange(128, dtype=np.float16), 32), (128, 128 * 32)).copy()
    common = dict(
        w0=W0.astype(bf16).view(np.uint16),
        wg=Wg.astype(bf16).view(np.uint16),
        wc1=np.asarray(Wc1, np.float32).astype(bf16).view(np.uint16),
        wc2=np.asarray(Wc2, np.float32).astype(bf16).view(np.uint16),
        b0=np.asarray(b0, np.float32).reshape(128, 1),
        bg=np.asarray(bg, np.float32).reshape(L, 128, 1),
        bc1=np.asarray(bc1, np.float32).reshape(128, 1),
        bc2m=np.broadcast_to(np.asarray(bc2, np.float32), (G, C)).copy(),
        stair=stair,
        id128=np.eye(128, dtype=np.float32).astype(bf16).view(np.uint16),
        idg=np.eye(G, dtype=np.float32).astype(bf16).view(np.uint16),
    )
    in_maps = []
    for c in range(NCORES):
        m = dict(common)
        m["xt"] = data["xt"][c].view(np.uint16)
        m["dinvt"] = data["dinvt"][c].view(np.uint16)
        m["gidx"] = data["gidx"][c]
        m["dstrel"] = data["dstrel"][c].astype(np.float16)
        m["dstrel_ex"] = data["dstrel_ex"][c].astype(np.float16)
        m["batchrel"] = data["batchrel"][c].astype(np.float16)
        in_maps.append(m)

    import os
    trace = os.environ.get("BASS_KERNEL_TRACE", "0") == "1"
    res = run_bass_kernel_spmd(nc, in_maps, list(range(NCORES)), trace=trace)
    kernel._last_exec_ns = res.exec_time_ns
    kernel._last_results = res
    return np.asarray(res.results[0]["out"], np.float32)


kernel._last_exec_ns = None
